# revision 2
# baseline (speedup 1.0000x reference)
"""Trainium2 Bass kernel for nn_AdaptiveLSTMCell2 (B=128, IN=256, H=256, AI=16, AH=128).

Strategy
--------
The reference materializes per-sample weight matrices m_w_ih/m_w_hh
(B x 256 x 1024 each, ~134 MB) from the hypernetwork and then does a batched
contraction.  We instead fuse the hypernetwork projection into the
contraction:

    igates[b,o] = sum_a h_ih[b,a] * (sum_i input_[b,i] * W2[(i,o),a])
                + sum_i input_[b,i] * b2[(i,o)]

The inner sum over i is a plain matmul with shared weights (PE-friendly);
the outer sum over a=16 is a small per-partition scaled accumulation (DVE).

Sharding: the 4H=1024 gate dimension is sharded 8 ways, interleaved so each
core owns h'-positions [32r, 32r+32) of all four gates.  Each core reads
only 1/8 of the two big hypernetwork projections (the only large tensors).
The outer LayerNorm needs global mean/var over all 1024 gate features: each
core computes bn_stats over its shard, a tiny AllGather shares the stats,
and bn_aggr recovers exact global stats.  The LSTM tail is then fully local;
the small adaptive LayerNormLSTMCell is computed redundantly on every core.

Matmul inputs are fed as bf16 (halves the HBM-bound weight traffic, full PE
rate); accumulation and all pointwise math stay f32.
"""
import os
import sys

for _p in ("/opt/trn_rl_repo", os.path.expanduser("~/.axon_site/_ro/trn_rl_repo")):
    if os.path.isdir(_p) and _p not in sys.path:
        sys.path.append(_p)

import numpy as np
import ml_dtypes

import concourse.bass as bass
import concourse.bacc as bacc
import concourse.tile as tile
from concourse import mybir
from concourse.bass_utils import run_bass_kernel_spmd

BF16 = ml_dtypes.bfloat16

B, IN, H, AI, AH = 128, 256, 256, 16, 128
M = 8            # cores
SH = H // M      # 32 h'-positions per core
EPS = 1e-5
NSLOT = 17       # 16 h-slots + 1 bias slot
SLOTW = 4 * SH   # 128 columns per slot (4 gates x 32)
W2W = NSLOT * SLOTW          # 2176
CHUNKS = [(0, 512), (512, 512), (1024, 512), (1536, 512), (2048, 128)]

# bcast vector layout (f32 offsets)
BC_AWI = 0       # a_ln_i_w   [512]
BC_AWH = 512     # a_ln_h_w   [512]
BC_AB = 1024     # a_ln_i_b + a_ln_h_b [512]
BC_CW = 1536     # a_ln_c_w   [128]
BC_CB = 1664     # a_ln_c_b   [128]
BC_W1B = 1792    # [w_ih1_b | w_hh1_b] [32]
BC_LNWI = 1824   # ln_i_w shard [128]
BC_LNWH = 1952   # ln_h_w shard [128]
BC_LNB = 2080    # (ln_i_b + ln_h_b) shard [128]
BC_N = 2208

_COMPILED = None
LAST_RESULT = None


def _build():
    f32 = mybir.dt.float32
    bf16 = mybir.dt.bfloat16
    nc = bacc.Bacc("TRN2", target_bir_lowering=False, debug=False, num_devices=M)

    ainT_e = nc.dram_tensor("ainT", [2 * IN, B], bf16, kind="ExternalInput")
    ahT_e = nc.dram_tensor("ahT", [AH, B], bf16, kind="ExternalInput")
    w1T_e = nc.dram_tensor("w1T", [AH, 2 * AI], f32, kind="ExternalInput")
    aWihT_e = nc.dram_tensor("aWihT", [2 * IN, 4 * AH], bf16, kind="ExternalInput")
    aWhhT_e = nc.dram_tensor("aWhhT", [AH, 4 * AH], bf16, kind="ExternalInput")
    w2c_e = nc.dram_tensor("w2c", [2, IN, W2W], bf16, kind="ExternalInput")
    nat_e = nc.dram_tensor("nat", [B, 288], f32, kind="ExternalInput")
    bc_e = nc.dram_tensor("bcast", [BC_N], f32, kind="ExternalInput")
    out_e = nc.dram_tensor("out", [B, 320], f32, kind="ExternalOutput")

    with tile.TileContext(nc, num_cores=M) as tc:
        import contextlib
        ctx = contextlib.ExitStack()
        with ctx:
            sb = ctx.enter_context(tc.tile_pool(name="sb", bufs=1))
            ps = ctx.enter_context(tc.tile_pool(name="ps", bufs=1, space="PSUM"))
            dr = ctx.enter_context(tc.tile_pool(name="dr", bufs=1, space="DRAM"))

            # ---------------- input DMAs (small first) ----------------
            ainT_sb = sb.tile([B, 4, B], bf16)
            nc.sync.dma_start(out=ainT_sb[:],
                              in_=ainT_e[:].rearrange("(t p) b -> p t b", p=B))
            ahT_sb = sb.tile([AH, B], bf16)
            nc.sync.dma_start(out=ahT_sb[:], in_=ahT_e[:])
            w1T_sb = sb.tile([AH, 2 * AI], f32)
            nc.sync.dma_start(out=w1T_sb[:], in_=w1T_e[:])
            nat_sb = sb.tile([B, 288], f32)
            nc.sync.dma_start(out=nat_sb[:], in_=nat_e[:])
            aWhhT_sb = sb.tile([AH, 4 * AH], bf16)
            nc.sync.dma_start(out=aWhhT_sb[:], in_=aWhhT_e[:])
            aWihT_sb = sb.tile([B, 4, 4 * AH], bf16)
            nc.sync.dma_start(out=aWihT_sb[:],
                              in_=aWihT_e[:].rearrange("(t p) j -> p t j", p=B))
            bc_sb = sb.tile([B, BC_N], f32)
            bc_ap = bass.AP(tensor=bc_e[:].tensor, offset=bc_e[:].offset,
                            ap=[[0, B], [1, BC_N]])
            nc.gpsimd.dma_start(out=bc_sb[:], in_=bc_ap)

            ident = nat_sb[:, 0:128]
            a_c_nat = nat_sb[:, 128:256]
            cx_nat = nat_sb[:, 256:288]

            # W2 chunk DMAs (the big streams; issued early, consumed later)
            w2t = {}
            for path in range(2):
                for ci, (c0, w) in enumerate(CHUNKS):
                    t = sb.tile([B, 2, w], bf16, name=f"w2t_{path}_{ci}",
                                tag="w2t", bufs=4)
                    nc.sync.dma_start(
                        out=t[:],
                        in_=w2c_e[:][path, :, c0:c0 + w].rearrange(
                            "(t p) c -> p t c", p=B))
                    w2t[(path, ci)] = t

            # ---------------- adaptive LayerNormLSTMCell ----------------
            ig_ps = ps.tile([B, 4 * AH], f32)
            for k in range(4):
                nc.tensor.matmul(ig_ps[:], ainT_sb[:, k, :], aWihT_sb[:, k, :],
                                 start=(k == 0), stop=(k == 3))
            hg_ps = ps.tile([B, 4 * AH], f32)
            nc.tensor.matmul(hg_ps[:], ahT_sb[:], aWhhT_sb[:],
                             start=True, stop=True)

            eps_t = sb.tile([B, 1], f32)
            nc.vector.memset(eps_t[:], EPS)

            def ln_stats(x_ap, name):
                st = sb.tile([B, 6], f32, name=f"st_{name}")
                nc.vector.bn_stats(out=st[:], in_=x_ap)
                mv = sb.tile([B, 2], f32, name=f"mv_{name}")
                nc.vector.bn_aggr(out=mv[:], in_=st[:])
                # mv[:,1] := 1/sqrt(var + eps)
                nc.scalar.activation(out=mv[:, 1:2], in_=mv[:, 1:2],
                                     func=mybir.ActivationFunctionType.Sqrt,
                                     bias=eps_t[:], scale=1.0)
                nc.vector.reciprocal(out=mv[:, 1:2], in_=mv[:, 1:2])
                return mv

            mv_ig = ln_stats(ig_ps[:], "aig")
            mv_hg = ln_stats(hg_ps[:], "ahg")

            tn_ig = sb.tile([B, 4 * AH], f32)
            nc.vector.tensor_scalar(out=tn_ig[:], in0=ig_ps[:],
                                    scalar1=mv_ig[:, 0:1], scalar2=mv_ig[:, 1:2],
                                    op0=mybir.AluOpType.subtract,
                                    op1=mybir.AluOpType.mult)
            tn_hg = sb.tile([B, 4 * AH], f32)
            nc.vector.tensor_scalar(out=tn_hg[:], in0=hg_ps[:],
                                    scalar1=mv_hg[:, 0:1], scalar2=mv_hg[:, 1:2],
                                    op0=mybir.AluOpType.subtract,
                                    op1=mybir.AluOpType.mult)
            # gate_in = tn_ig*w_i + tn_hg*w_h + (b_i + b_h)
            g1 = sb.tile([B, 4 * AH], f32)
            nc.vector.tensor_mul(out=g1[:], in0=tn_ig[:],
                                 in1=bc_sb[:, BC_AWI:BC_AWI + 512])
            g2 = sb.tile([B, 4 * AH], f32)
            nc.vector.tensor_mul(out=g2[:], in0=tn_hg[:],
                                 in1=bc_sb[:, BC_AWH:BC_AWH + 512])
            nc.vector.tensor_add(out=g1[:], in0=g1[:], in1=g2[:])
            nc.vector.tensor_add(out=g1[:], in0=g1[:],
                                 in1=bc_sb[:, BC_AB:BC_AB + 512])

            asig = sb.tile([B, 2 * AH], f32)     # sigmoid(gi) | sigmoid(gf)
            nc.scalar.activation(out=asig[:], in_=g1[:, 0:256],
                                 func=mybir.ActivationFunctionType.Sigmoid)
            atanh = sb.tile([B, AH], f32)        # tanh(gc)
            nc.scalar.activation(out=atanh[:], in_=g1[:, 256:384],
                                 func=mybir.ActivationFunctionType.Tanh)
            asig_o = sb.tile([B, AH], f32)       # sigmoid(go)
            nc.scalar.activation(out=asig_o[:], in_=g1[:, 384:512],
                                 func=mybir.ActivationFunctionType.Sigmoid)

            t_m1 = sb.tile([B, AH], f32)
            nc.vector.tensor_mul(out=t_m1[:], in0=asig[:, 128:256], in1=a_c_nat)
            t_m2 = sb.tile([B, AH], f32)
            nc.vector.tensor_mul(out=t_m2[:], in0=asig[:, 0:128], in1=atanh[:])
            cpre = sb.tile([B, AH], f32)
            nc.vector.tensor_add(out=cpre[:], in0=t_m1[:], in1=t_m2[:])

            mv_c = ln_stats(cpre[:], "ac")
            cn = sb.tile([B, AH], f32)
            nc.vector.tensor_scalar(out=cn[:], in0=cpre[:],
                                    scalar1=mv_c[:, 0:1], scalar2=mv_c[:, 1:2],
                                    op0=mybir.AluOpType.subtract,
                                    op1=mybir.AluOpType.mult)

            out_sb = sb.tile([B, 320], f32)
            # a_c_new -> out[:, 192:320]
            nc.vector.tensor_mul(out=cn[:], in0=cn[:],
                                 in1=bc_sb[:, BC_CW:BC_CW + 128])
            nc.vector.tensor_add(out=out_sb[:, 192:320], in0=cn[:],
                                 in1=bc_sb[:, BC_CB:BC_CB + 128])
            tacn = sb.tile([B, AH], f32)
            nc.scalar.activation(out=tacn[:], in_=out_sb[:, 192:320],
                                 func=mybir.ActivationFunctionType.Tanh)
            # a_h_new -> out[:, 64:192]
            nc.vector.tensor_mul(out=out_sb[:, 64:192], in0=asig_o[:],
                                 in1=tacn[:])

            # h = [h_ih | h_hh] = a_h_new @ [w_ih1_W.T | w_hh1_W.T] + b
            tr_ps = ps.tile([B, B], f32)
            nc.tensor.transpose(tr_ps[:], out_sb[:, 64:192], ident)
            ahnT_sb = sb.tile([B, B], f32)
            nc.scalar.copy(out=ahnT_sb[:], in_=tr_ps[:])
            h_ps = ps.tile([B, 2 * AI], f32)
            nc.tensor.matmul(h_ps[:], ahnT_sb[:], w1T_sb[:],
                             start=True, stop=True)
            h_sb = sb.tile([B, 2 * AI], f32)
            nc.vector.tensor_add(out=h_sb[:], in0=h_ps[:],
                                 in1=bc_sb[:, BC_W1B:BC_W1B + 32])

            # ---------------- fused hypernetwork contraction ----------------
            accs = []
            for path in range(2):
                acc = sb.tile([B, SLOTW], f32, name=f"acc_{path}")
                for ci, (c0, w) in enumerate(CHUNKS):
                    mm = ps.tile([B, w], f32, name=f"mm_{path}_{ci}",
                                 tag="mm", bufs=4)
                    for k in range(2):
                        nc.tensor.matmul(mm[:], ainT_sb[:, 2 * path + k, :],
                                         w2t[(path, ci)][:, k, :],
                                         start=(k == 0), stop=(k == 1))
                    for j in range(w // SLOTW):
                        a = ci * 4 + j
                        src = mm[:, j * SLOTW:(j + 1) * SLOTW]
                        hcol = h_sb[:, AI * path + min(a, 15):
                                    AI * path + min(a, 15) + 1]
                        if a == 0:
                            nc.vector.tensor_scalar_mul(out=acc[:], in0=src,
                                                        scalar1=hcol)
                        elif a < 16:
                            nc.vector.scalar_tensor_tensor(
                                out=acc[:], in0=src, scalar=hcol, in1=acc[:],
                                op0=mybir.AluOpType.mult,
                                op1=mybir.AluOpType.add)
                        else:  # bias slot
                            nc.vector.tensor_add(out=acc[:], in0=acc[:], in1=src)
                accs.append(acc)

            # ---------------- distributed LayerNorm stats ----------------
            stats_sb = sb.tile([B, 12], f32)
            nc.vector.bn_stats(out=stats_sb[:, 0:6], in_=accs[0][:])
            nc.vector.bn_stats(out=stats_sb[:, 6:12], in_=accs[1][:])

            ag_in = dr.tile([B, 12], f32)
            ag_out = dr.tile([M, B, 12], f32)
            nc.sync.dma_start(out=ag_in[:], in_=stats_sb[:])
            nc.gpsimd.collective_compute(
                "AllGather", mybir.AluOpType.bypass,
                replica_groups=[list(range(M))],
                ins=[ag_in[:].opt()], outs=[ag_out[:].opt()])

            allst_ig = sb.tile([B, M, 6], f32)
            nc.sync.dma_start(out=allst_ig[:],
                              in_=ag_out[:][:, :, 0:6].rearrange("r p c -> p r c"))
            allst_hg = sb.tile([B, M, 6], f32)
            nc.sync.dma_start(out=allst_hg[:],
                              in_=ag_out[:][:, :, 6:12].rearrange("r p c -> p r c"))

            def g_stats(allst, name):
                mv = sb.tile([B, 2], f32, name=f"gmv_{name}")
                nc.vector.bn_aggr(out=mv[:], in_=allst[:])
                nc.scalar.activation(out=mv[:, 1:2], in_=mv[:, 1:2],
                                     func=mybir.ActivationFunctionType.Sqrt,
                                     bias=eps_t[:], scale=1.0)
                nc.vector.reciprocal(out=mv[:, 1:2], in_=mv[:, 1:2])
                return mv

            gmv_ig = g_stats(allst_ig, "ig")
            gmv_hg = g_stats(allst_hg, "hg")

            # gates = LN(ig)*w_i + b_i + LN(hg)*w_h + b_h  (shard slice)
            q1 = sb.tile([B, SLOTW], f32)
            nc.vector.tensor_scalar(out=q1[:], in0=accs[0][:],
                                    scalar1=gmv_ig[:, 0:1], scalar2=gmv_ig[:, 1:2],
                                    op0=mybir.AluOpType.subtract,
                                    op1=mybir.AluOpType.mult)
            q2 = sb.tile([B, SLOTW], f32)
            nc.vector.tensor_scalar(out=q2[:], in0=accs[1][:],
                                    scalar1=gmv_hg[:, 0:1], scalar2=gmv_hg[:, 1:2],
                                    op0=mybir.AluOpType.subtract,
                                    op1=mybir.AluOpType.mult)
            nc.vector.tensor_mul(out=q1[:], in0=q1[:],
                                 in1=bc_sb[:, BC_LNWI:BC_LNWI + 128])
            nc.vector.tensor_mul(out=q2[:], in0=q2[:],
                                 in1=bc_sb[:, BC_LNWH:BC_LNWH + 128])
            nc.vector.tensor_add(out=q1[:], in0=q1[:], in1=q2[:])
            nc.vector.tensor_add(out=q1[:], in0=q1[:],
                                 in1=bc_sb[:, BC_LNB:BC_LNB + 128])

            gs = sb.tile([B, SLOTW], f32)
            nc.scalar.activation(out=gs[:, 0:64], in_=q1[:, 0:64],
                                 func=mybir.ActivationFunctionType.Sigmoid)
            nc.scalar.activation(out=gs[:, 64:96], in_=q1[:, 64:96],
                                 func=mybir.ActivationFunctionType.Tanh)
            nc.scalar.activation(out=gs[:, 96:128], in_=q1[:, 96:128],
                                 func=mybir.ActivationFunctionType.Sigmoid)

            f_m1 = sb.tile([B, SH], f32)
            nc.vector.tensor_mul(out=f_m1[:], in0=gs[:, 32:64], in1=cx_nat)
            f_m2 = sb.tile([B, SH], f32)
            nc.vector.tensor_mul(out=f_m2[:], in0=gs[:, 0:32], in1=gs[:, 64:96])
            # cy -> out[:, 32:64]
            nc.vector.tensor_add(out=out_sb[:, 32:64], in0=f_m1[:], in1=f_m2[:])
            tcy = sb.tile([B, SH], f32)
            nc.scalar.activation(out=tcy[:], in_=out_sb[:, 32:64],
                                 func=mybir.ActivationFunctionType.Tanh)
            # hy -> out[:, 0:32]
            nc.vector.tensor_mul(out=out_sb[:, 0:32], in0=gs[:, 96:128],
                                 in1=tcy[:])

            nc.sync.dma_start(out=out_e[:], in_=out_sb[:])

    nc.compile()
    return nc


def _prep_inputs(inp):
    """Host-side layout prep: sharding, transposes, packing. No FLOPs beyond
    layout transforms and additive bias pre-combination."""
    f = lambda k: np.ascontiguousarray(np.asarray(inp[k], dtype=np.float32))
    input_, total_h, total_c = f("input_"), f("total_h"), f("total_c")
    hx, a_h = total_h[:, :H], total_h[:, H:]
    cx, a_c = total_c[:, :H], total_c[:, H:]

    ainT = np.concatenate([input_.T, hx.T], axis=0)              # [512, B]
    ahT = np.ascontiguousarray(a_h.T)                            # [128, B]
    w1T = np.concatenate([f("w_ih1_W").T, f("w_hh1_W").T], 1)    # [128, 32]
    aWihT = np.ascontiguousarray(f("a_w_ih").T)                  # [512, 512]
    aWhhT = np.ascontiguousarray(f("a_w_hh").T)                  # [128, 512]

    bc_common = np.concatenate([
        f("a_ln_i_w"), f("a_ln_h_w"), f("a_ln_i_b") + f("a_ln_h_b"),
        f("a_ln_c_w"), f("a_ln_c_b"),
        np.concatenate([f("w_ih1_b"), f("w_hh1_b")]),
    ])
    lnw_i = f("ln_i_w").reshape(4, H); lnb_i = f("ln_i_b").reshape(4, H)
    lnw_h = f("ln_h_w").reshape(4, H); lnb_h = f("ln_h_b").reshape(4, H)

    W2 = [f("w_ih2_W").reshape(IN, 4, H, AI), f("w_hh2_W").reshape(H, 4, H, AI)]
    B2 = [f("w_ih2_b").reshape(IN, 4, H), f("w_hh2_b").reshape(H, 4, H)]

    in_maps = []
    for r in range(M):
        sl = slice(SH * r, SH * (r + 1))
        w2c = np.empty((2, IN, W2W), dtype=BF16)
        for p in range(2):
            w2c[p, :, :2048] = W2[p][:, :, sl, :].transpose(0, 3, 1, 2).reshape(IN, 2048)
            w2c[p, :, 2048:] = B2[p][:, :, sl].reshape(IN, 128)
        nat = np.concatenate([np.eye(B, dtype=np.float32), a_c,
                              cx[:, sl]], axis=1)
        bc = np.concatenate([
            bc_common,
            lnw_i[:, sl].reshape(-1), lnw_h[:, sl].reshape(-1),
            (lnb_i[:, sl] + lnb_h[:, sl]).reshape(-1),
        ]).astype(np.float32)
        in_maps.append({
            "ainT": ainT.astype(BF16), "ahT": ahT.astype(BF16),
            "w1T": w1T.astype(np.float32),
            "aWihT": aWihT.astype(BF16), "aWhhT": aWhhT.astype(BF16),
            "w2c": w2c, "nat": nat.astype(np.float32), "bcast": bc,
        })
    return in_maps


def kernel(**inputs):
    global _COMPILED, LAST_RESULT
    if _COMPILED is None:
        _COMPILED = _build()
    in_maps = _prep_inputs(inputs)
    res = run_bass_kernel_spmd(_COMPILED, in_maps, core_ids=list(range(M)))
    LAST_RESULT = res

    hy = np.empty((B, H), np.float32)
    cy = np.empty((B, H), np.float32)
    for r in range(M):
        o = res.results[r]["out"]
        hy[:, SH * r:SH * (r + 1)] = o[:, 0:32]
        cy[:, SH * r:SH * (r + 1)] = o[:, 32:64]
    o0 = res.results[0]["out"]
    new_total_h = np.concatenate([hy, o0[:, 64:192]], axis=1)
    new_total_c = np.concatenate([cy, o0[:, 192:320]], axis=1)
    return hy, new_total_h, new_total_c


# revision 3
# speedup vs baseline: 1.0096x; 1.0096x over previous
"""Trainium2 Bass kernel for nn_AdaptiveLSTMCell2 (B=128, IN=256, H=256, AI=16, AH=128).

Strategy
--------
The reference materializes per-sample weight matrices m_w_ih/m_w_hh
(B x 256 x 1024 each, ~134 MB) from the hypernetwork and then does a batched
contraction.  We instead fuse the hypernetwork projection into the
contraction:

    igates[b,o] = sum_a h_ih[b,a] * (sum_i input_[b,i] * W2[(i,o),a])
                + sum_i input_[b,i] * b2[(i,o)]

The inner sum over i is a plain matmul with shared weights (PE-friendly, bf16);
the outer sum over a=16 is a small per-partition scaled accumulation (DVE, f32).

Sharding: the 4H=1024 gate dimension is sharded 8 ways, interleaved so each
core owns h'-positions [32r, 32r+32) of all four gates.  Each core reads
only 1/8 of the two big hypernetwork projections (the only large tensors).
The outer LayerNorm needs global mean/var over all 1024 gate features: each
core computes bn_stats over its shard, a tiny AllGather shares the stats,
and bn_aggr recovers exact global stats.  The LSTM tail is then fully local;
the small adaptive LayerNormLSTMCell is computed redundantly on every core
in f32.

All inputs are host-repacked into per-partition-contiguous slabs so every
DMA moves large contiguous runs.
"""
import os
import sys

for _p in ("/opt/trn_rl_repo", os.path.expanduser("~/.axon_site/_ro/trn_rl_repo")):
    if os.path.isdir(_p) and _p not in sys.path:
        sys.path.append(_p)

import numpy as np
import ml_dtypes

import concourse.bass as bass
import concourse.bacc as bacc
import concourse.tile as tile
from concourse import mybir
from concourse.bass_utils import run_bass_kernel_spmd

BF16 = ml_dtypes.bfloat16

B, IN, H, AI, AH = 128, 256, 256, 16, 128
M = 8            # cores
SH = H // M      # 32 h'-positions per core
EPS = 1e-5
NSLOT = 17       # 16 h-slots + 1 bias slot
SLOTW = 4 * SH   # 128 columns per slot (4 gates x 32)
W2W = NSLOT * SLOTW          # 2176
CHUNKS = [(0, 512), (512, 512), (1024, 512), (1536, 512), (2048, 128)]

# slab32 element offsets (f32, per partition)
S_AINT = 0       # a_in^T tiles (4 x 128)          [0:512)
S_AWIH = 512     # a_w_ih^T tiles (4 x 512)        [512:2560)
S_AWHH = 2560    # a_w_hh^T [128, 512]             [2560:3072)
S_AHT = 3072     # a_h^T [128, 128]                [3072:3200)
S_ID = 3200      # identity [128, 128]             [3200:3328)
S_AC = 3328      # a_c [128, 128]                  [3328:3456)
S_CX = 3456      # cx shard [128, 32]              [3456:3488)
S_W1T = 3488     # [w_ih1^T | w_hh1^T] [128, 32]   [3488:3520)
S32 = 3520

# slab16 element offsets (bf16, per partition)
S_AINT16 = 0     # a_in^T tiles bf16 (4 x 128)     [0:512)
S_W2C = 512      # then 10 w2c blocks of 2*w each
_off = S_W2C
W2OFF = {}
for _path in range(2):
    for _ci, (_c0, _w) in enumerate(CHUNKS):
        W2OFF[(_path, _ci)] = _off
        _off += 2 * _w
S16 = _off       # 9216

# bcast vector layout (f32 offsets)
BC_AWI = 0       # a_ln_i_w   [512]
BC_AWH = 512     # a_ln_h_w   [512]
BC_AB = 1024     # a_ln_i_b + a_ln_h_b [512]
BC_CW = 1536     # a_ln_c_w   [128]
BC_CB = 1664     # a_ln_c_b   [128]
BC_W1B = 1792    # [w_ih1_b | w_hh1_b] [32]
BC_LNWI = 1824   # ln_i_w shard [128]
BC_LNWH = 1952   # ln_h_w shard [128]
BC_LNB = 2080    # (ln_i_b + ln_h_b) shard [128]
BC_N = 2208

_COMPILED = None
LAST_RESULT = None


def _build():
    f32 = mybir.dt.float32
    bf16 = mybir.dt.bfloat16
    nc = bacc.Bacc("TRN2", target_bir_lowering=False, debug=False, num_devices=M)

    s32_e = nc.dram_tensor("slab32", [B, S32], f32, kind="ExternalInput")
    s16_e = nc.dram_tensor("slab16", [B, S16], bf16, kind="ExternalInput")
    bc_e = nc.dram_tensor("bcast", [BC_N], f32, kind="ExternalInput")
    out_e = nc.dram_tensor("out", [B, 320], f32, kind="ExternalOutput")

    with tile.TileContext(nc, num_cores=M) as tc:
        import contextlib
        ctx = contextlib.ExitStack()
        with ctx:
            sb = ctx.enter_context(tc.tile_pool(name="sb", bufs=1))
            ps = ctx.enter_context(tc.tile_pool(name="ps", bufs=1, space="PSUM"))
            dr = ctx.enter_context(tc.tile_pool(name="dr", bufs=1, space="DRAM"))

            # ---------------- input DMAs ----------------
            s32_sb = sb.tile([B, S32], f32)
            nc.sync.dma_start(out=s32_sb[:, 0:S_AHT],
                              in_=s32_e[:][:, 0:S_AHT])
            nc.sync.dma_start(out=s32_sb[:, S_AHT:S32],
                              in_=s32_e[:][:, S_AHT:S32])
            s16_sb = sb.tile([B, S16], bf16)
            nc.sync.dma_start(out=s16_sb[:, 0:S_W2C],
                              in_=s16_e[:][:, 0:S_W2C])
            bc_sb = sb.tile([B, BC_N], f32)
            bc_ap = bass.AP(tensor=bc_e[:].tensor, offset=bc_e[:].offset,
                            ap=[[0, B], [1, BC_N]])
            nc.gpsimd.dma_start(out=bc_sb[:], in_=bc_ap)
            for path in range(2):
                for ci, (c0, w) in enumerate(CHUNKS):
                    o = W2OFF[(path, ci)]
                    nc.sync.dma_start(out=s16_sb[:, o:o + 2 * w],
                                      in_=s16_e[:][:, o:o + 2 * w])

            ainT32 = s32_sb[:, S_AINT:S_AINT + 512].rearrange(
                "p (t b) -> p t b", t=4)
            aWihT = s32_sb[:, S_AWIH:S_AWIH + 2048].rearrange(
                "p (t j) -> p t j", t=4)
            aWhhT = s32_sb[:, S_AWHH:S_AWHH + 512]
            ahT = s32_sb[:, S_AHT:S_AHT + 128]
            ident = s32_sb[:, S_ID:S_ID + 128]
            a_c_nat = s32_sb[:, S_AC:S_AC + 128]
            cx_nat = s32_sb[:, S_CX:S_CX + 32]
            w1T = s32_sb[:, S_W1T:S_W1T + 32]
            ainT16 = s16_sb[:, S_AINT16:S_AINT16 + 512].rearrange(
                "p (t b) -> p t b", t=4)

            # ---------------- adaptive LayerNormLSTMCell (f32) ----------------
            ig_ps = ps.tile([B, 4 * AH], f32)
            for k in range(4):
                nc.tensor.matmul(ig_ps[:], ainT32[:, k, :], aWihT[:, k, :],
                                 start=(k == 0), stop=(k == 3))
            hg_ps = ps.tile([B, 4 * AH], f32)
            nc.tensor.matmul(hg_ps[:], ahT, aWhhT,
                             start=True, stop=True)

            eps_t = sb.tile([B, 1], f32)
            nc.vector.memset(eps_t[:], EPS)

            def ln_stats(x_ap, name):
                st = sb.tile([B, 6], f32, name=f"st_{name}")
                nc.vector.bn_stats(out=st[:], in_=x_ap)
                mv = sb.tile([B, 2], f32, name=f"mv_{name}")
                nc.vector.bn_aggr(out=mv[:], in_=st[:])
                # mv[:,1] := 1/sqrt(var + eps)
                nc.scalar.activation(out=mv[:, 1:2], in_=mv[:, 1:2],
                                     func=mybir.ActivationFunctionType.Sqrt,
                                     bias=eps_t[:], scale=1.0)
                nc.vector.reciprocal(out=mv[:, 1:2], in_=mv[:, 1:2])
                return mv

            mv_ig = ln_stats(ig_ps[:], "aig")
            mv_hg = ln_stats(hg_ps[:], "ahg")

            tn_ig = sb.tile([B, 4 * AH], f32)
            nc.vector.tensor_scalar(out=tn_ig[:], in0=ig_ps[:],
                                    scalar1=mv_ig[:, 0:1], scalar2=mv_ig[:, 1:2],
                                    op0=mybir.AluOpType.subtract,
                                    op1=mybir.AluOpType.mult)
            tn_hg = sb.tile([B, 4 * AH], f32)
            nc.vector.tensor_scalar(out=tn_hg[:], in0=hg_ps[:],
                                    scalar1=mv_hg[:, 0:1], scalar2=mv_hg[:, 1:2],
                                    op0=mybir.AluOpType.subtract,
                                    op1=mybir.AluOpType.mult)
            # gate_in = tn_ig*w_i + tn_hg*w_h + (b_i + b_h)
            g1 = sb.tile([B, 4 * AH], f32)
            nc.vector.tensor_mul(out=g1[:], in0=tn_ig[:],
                                 in1=bc_sb[:, BC_AWI:BC_AWI + 512])
            g2 = sb.tile([B, 4 * AH], f32)
            nc.vector.tensor_mul(out=g2[:], in0=tn_hg[:],
                                 in1=bc_sb[:, BC_AWH:BC_AWH + 512])
            nc.vector.tensor_add(out=g1[:], in0=g1[:], in1=g2[:])
            nc.vector.tensor_add(out=g1[:], in0=g1[:],
                                 in1=bc_sb[:, BC_AB:BC_AB + 512])

            asig = sb.tile([B, 2 * AH], f32)     # sigmoid(gi) | sigmoid(gf)
            nc.scalar.activation(out=asig[:], in_=g1[:, 0:256],
                                 func=mybir.ActivationFunctionType.Sigmoid)
            atanh = sb.tile([B, AH], f32)        # tanh(gc)
            nc.scalar.activation(out=atanh[:], in_=g1[:, 256:384],
                                 func=mybir.ActivationFunctionType.Tanh)
            asig_o = sb.tile([B, AH], f32)       # sigmoid(go)
            nc.scalar.activation(out=asig_o[:], in_=g1[:, 384:512],
                                 func=mybir.ActivationFunctionType.Sigmoid)

            t_m1 = sb.tile([B, AH], f32)
            nc.vector.tensor_mul(out=t_m1[:], in0=asig[:, 128:256], in1=a_c_nat)
            t_m2 = sb.tile([B, AH], f32)
            nc.vector.tensor_mul(out=t_m2[:], in0=asig[:, 0:128], in1=atanh[:])
            cpre = sb.tile([B, AH], f32)
            nc.vector.tensor_add(out=cpre[:], in0=t_m1[:], in1=t_m2[:])

            mv_c = ln_stats(cpre[:], "ac")
            cn = sb.tile([B, AH], f32)
            nc.vector.tensor_scalar(out=cn[:], in0=cpre[:],
                                    scalar1=mv_c[:, 0:1], scalar2=mv_c[:, 1:2],
                                    op0=mybir.AluOpType.subtract,
                                    op1=mybir.AluOpType.mult)

            out_sb = sb.tile([B, 320], f32)
            # a_c_new -> out[:, 192:320]
            nc.vector.tensor_mul(out=cn[:], in0=cn[:],
                                 in1=bc_sb[:, BC_CW:BC_CW + 128])
            nc.vector.tensor_add(out=out_sb[:, 192:320], in0=cn[:],
                                 in1=bc_sb[:, BC_CB:BC_CB + 128])
            tacn = sb.tile([B, AH], f32)
            nc.scalar.activation(out=tacn[:], in_=out_sb[:, 192:320],
                                 func=mybir.ActivationFunctionType.Tanh)
            # a_h_new -> out[:, 64:192]
            nc.vector.tensor_mul(out=out_sb[:, 64:192], in0=asig_o[:],
                                 in1=tacn[:])
            # adaptive outputs can leave early
            nc.sync.dma_start(out=out_e[:][:, 64:320], in_=out_sb[:, 64:320])

            # h = [h_ih | h_hh] = a_h_new @ [w_ih1_W.T | w_hh1_W.T] + b
            tr_ps = ps.tile([B, B], f32)
            nc.tensor.transpose(tr_ps[:], out_sb[:, 64:192], ident)
            ahnT_sb = sb.tile([B, B], f32)
            nc.scalar.copy(out=ahnT_sb[:], in_=tr_ps[:])
            h_ps = ps.tile([B, 2 * AI], f32)
            nc.tensor.matmul(h_ps[:], ahnT_sb[:], w1T,
                             start=True, stop=True)
            h_sb = sb.tile([B, 2 * AI], f32)
            nc.vector.tensor_add(out=h_sb[:], in0=h_ps[:],
                                 in1=bc_sb[:, BC_W1B:BC_W1B + 32])

            # ---------------- fused hypernetwork contraction ----------------
            accs = []
            for path in range(2):
                acc = sb.tile([B, SLOTW], f32, name=f"acc_{path}")
                for ci, (c0, w) in enumerate(CHUNKS):
                    w2v = s16_sb[:, W2OFF[(path, ci)]:
                                 W2OFF[(path, ci)] + 2 * w].rearrange(
                                     "p (t c) -> p t c", t=2)
                    mm = ps.tile([B, w], f32, name=f"mm_{path}_{ci}",
                                 tag="mm", bufs=4)
                    for k in range(2):
                        nc.tensor.matmul(mm[:], ainT16[:, 2 * path + k, :],
                                         w2v[:, k, :],
                                         start=(k == 0), stop=(k == 1))
                    for j in range(w // SLOTW):
                        a = ci * 4 + j
                        src = mm[:, j * SLOTW:(j + 1) * SLOTW]
                        hcol = h_sb[:, AI * path + min(a, 15):
                                    AI * path + min(a, 15) + 1]
                        if a == 0:
                            nc.vector.tensor_scalar_mul(out=acc[:], in0=src,
                                                        scalar1=hcol)
                        elif a < 16:
                            nc.vector.scalar_tensor_tensor(
                                out=acc[:], in0=src, scalar=hcol, in1=acc[:],
                                op0=mybir.AluOpType.mult,
                                op1=mybir.AluOpType.add)
                        else:  # bias slot
                            nc.vector.tensor_add(out=acc[:], in0=acc[:], in1=src)
                accs.append(acc)

            # ---------------- distributed LayerNorm stats ----------------
            stats_sb = sb.tile([B, 12], f32)
            nc.vector.bn_stats(out=stats_sb[:, 0:6], in_=accs[0][:])
            nc.vector.bn_stats(out=stats_sb[:, 6:12], in_=accs[1][:])

            ag_in = dr.tile([B, 12], f32)
            ag_out = dr.tile([M, B, 12], f32)
            nc.sync.dma_start(out=ag_in[:], in_=stats_sb[:])
            nc.gpsimd.collective_compute(
                "AllGather", mybir.AluOpType.bypass,
                replica_groups=[list(range(M))],
                ins=[ag_in[:].opt()], outs=[ag_out[:].opt()])

            allst_ig = sb.tile([B, M, 6], f32)
            nc.sync.dma_start(out=allst_ig[:],
                              in_=ag_out[:][:, :, 0:6].rearrange("r p c -> p r c"))
            allst_hg = sb.tile([B, M, 6], f32)
            nc.sync.dma_start(out=allst_hg[:],
                              in_=ag_out[:][:, :, 6:12].rearrange("r p c -> p r c"))

            def g_stats(allst, name):
                mv = sb.tile([B, 2], f32, name=f"gmv_{name}")
                nc.vector.bn_aggr(out=mv[:], in_=allst[:])
                nc.scalar.activation(out=mv[:, 1:2], in_=mv[:, 1:2],
                                     func=mybir.ActivationFunctionType.Sqrt,
                                     bias=eps_t[:], scale=1.0)
                nc.vector.reciprocal(out=mv[:, 1:2], in_=mv[:, 1:2])
                return mv

            gmv_ig = g_stats(allst_ig, "ig")
            gmv_hg = g_stats(allst_hg, "hg")

            # gates = LN(ig)*w_i + b_i + LN(hg)*w_h + b_h  (shard slice)
            q1 = sb.tile([B, SLOTW], f32)
            nc.vector.tensor_scalar(out=q1[:], in0=accs[0][:],
                                    scalar1=gmv_ig[:, 0:1], scalar2=gmv_ig[:, 1:2],
                                    op0=mybir.AluOpType.subtract,
                                    op1=mybir.AluOpType.mult)
            q2 = sb.tile([B, SLOTW], f32)
            nc.vector.tensor_scalar(out=q2[:], in0=accs[1][:],
                                    scalar1=gmv_hg[:, 0:1], scalar2=gmv_hg[:, 1:2],
                                    op0=mybir.AluOpType.subtract,
                                    op1=mybir.AluOpType.mult)
            nc.vector.tensor_mul(out=q1[:], in0=q1[:],
                                 in1=bc_sb[:, BC_LNWI:BC_LNWI + 128])
            nc.vector.tensor_mul(out=q2[:], in0=q2[:],
                                 in1=bc_sb[:, BC_LNWH:BC_LNWH + 128])
            nc.vector.tensor_add(out=q1[:], in0=q1[:], in1=q2[:])
            nc.vector.tensor_add(out=q1[:], in0=q1[:],
                                 in1=bc_sb[:, BC_LNB:BC_LNB + 128])

            gs = sb.tile([B, SLOTW], f32)
            nc.scalar.activation(out=gs[:, 0:64], in_=q1[:, 0:64],
                                 func=mybir.ActivationFunctionType.Sigmoid)
            nc.scalar.activation(out=gs[:, 64:96], in_=q1[:, 64:96],
                                 func=mybir.ActivationFunctionType.Tanh)
            nc.scalar.activation(out=gs[:, 96:128], in_=q1[:, 96:128],
                                 func=mybir.ActivationFunctionType.Sigmoid)

            f_m1 = sb.tile([B, SH], f32)
            nc.vector.tensor_mul(out=f_m1[:], in0=gs[:, 32:64], in1=cx_nat)
            f_m2 = sb.tile([B, SH], f32)
            nc.vector.tensor_mul(out=f_m2[:], in0=gs[:, 0:32], in1=gs[:, 64:96])
            # cy -> out[:, 32:64]
            nc.vector.tensor_add(out=out_sb[:, 32:64], in0=f_m1[:], in1=f_m2[:])
            tcy = sb.tile([B, SH], f32)
            nc.scalar.activation(out=tcy[:], in_=out_sb[:, 32:64],
                                 func=mybir.ActivationFunctionType.Tanh)
            # hy -> out[:, 0:32]
            nc.vector.tensor_mul(out=out_sb[:, 0:32], in0=gs[:, 96:128],
                                 in1=tcy[:])

            nc.sync.dma_start(out=out_e[:][:, 0:64], in_=out_sb[:, 0:64])

    nc.compile()
    return nc


def _prep_inputs(inp):
    """Host-side layout prep: sharding, transposes, packing. No FLOPs beyond
    layout transforms and additive bias pre-combination."""
    f = lambda k: np.ascontiguousarray(np.asarray(inp[k], dtype=np.float32))
    input_, total_h, total_c = f("input_"), f("total_h"), f("total_c")
    hx, a_h = total_h[:, :H], total_h[:, H:]
    cx, a_c = total_c[:, :H], total_c[:, H:]

    # [p, (t, x)] packings
    ainT = np.concatenate([input_.T, hx.T], 0).reshape(4, B, B) \
        .transpose(1, 0, 2).reshape(B, 512)                      # [128, 512]
    aWihT = f("a_w_ih").T.reshape(4, B, 512) \
        .transpose(1, 0, 2).reshape(B, 2048)                     # [128, 2048]
    aWhhT = f("a_w_hh").T                                        # [128, 512]
    ahT = np.ascontiguousarray(a_h.T)                            # [128, 128]
    w1T = np.concatenate([f("w_ih1_W").T, f("w_hh1_W").T], 1)    # [128, 32]

    bc_common = np.concatenate([
        f("a_ln_i_w"), f("a_ln_h_w"), f("a_ln_i_b") + f("a_ln_h_b"),
        f("a_ln_c_w"), f("a_ln_c_b"),
        np.concatenate([f("w_ih1_b"), f("w_hh1_b")]),
    ])
    lnw_i = f("ln_i_w").reshape(4, H); lnb_i = f("ln_i_b").reshape(4, H)
    lnw_h = f("ln_h_w").reshape(4, H); lnb_h = f("ln_h_b").reshape(4, H)

    W2 = [f("w_ih2_W").reshape(IN, 4, H, AI), f("w_hh2_W").reshape(H, 4, H, AI)]
    B2 = [f("w_ih2_b").reshape(IN, 4, H), f("w_hh2_b").reshape(H, 4, H)]

    in_maps = []
    for r in range(M):
        sl = slice(SH * r, SH * (r + 1))
        slab32 = np.empty((B, S32), np.float32)
        slab32[:, S_AINT:S_AINT + 512] = ainT
        slab32[:, S_AWIH:S_AWIH + 2048] = aWihT
        slab32[:, S_AWHH:S_AWHH + 512] = aWhhT
        slab32[:, S_AHT:S_AHT + 128] = ahT
        slab32[:, S_ID:S_ID + 128] = np.eye(B, dtype=np.float32)
        slab32[:, S_AC:S_AC + 128] = a_c
        slab32[:, S_CX:S_CX + 32] = cx[:, sl]
        slab32[:, S_W1T:S_W1T + 32] = w1T

        slab16 = np.empty((B, S16), BF16)
        slab16[:, S_AINT16:S_AINT16 + 512] = ainT.astype(BF16)
        for p in range(2):
            w2p = np.empty((IN, W2W), np.float32)
            w2p[:, :2048] = W2[p][:, :, sl, :].transpose(0, 3, 1, 2) \
                .reshape(IN, 2048)
            w2p[:, 2048:] = B2[p][:, :, sl].reshape(IN, 128)
            for ci, (c0, w) in enumerate(CHUNKS):
                o = W2OFF[(p, ci)]
                slab16[:, o:o + 2 * w] = w2p[:, c0:c0 + w] \
                    .reshape(2, B, w).transpose(1, 0, 2) \
                    .reshape(B, 2 * w).astype(BF16)

        bc = np.concatenate([
            bc_common,
            lnw_i[:, sl].reshape(-1), lnw_h[:, sl].reshape(-1),
            (lnb_i[:, sl] + lnb_h[:, sl]).reshape(-1),
        ]).astype(np.float32)
        in_maps.append({"slab32": slab32, "slab16": slab16, "bcast": bc})
    return in_maps


def kernel(**inputs):
    global _COMPILED, LAST_RESULT
    if _COMPILED is None:
        _COMPILED = _build()
    in_maps = _prep_inputs(inputs)
    res = run_bass_kernel_spmd(_COMPILED, in_maps, core_ids=list(range(M)))
    LAST_RESULT = res

    hy = np.empty((B, H), np.float32)
    cy = np.empty((B, H), np.float32)
    for r in range(M):
        o = res.results[r]["out"]
        hy[:, SH * r:SH * (r + 1)] = o[:, 0:32]
        cy[:, SH * r:SH * (r + 1)] = o[:, 32:64]
    o0 = res.results[0]["out"]
    new_total_h = np.concatenate([hy, o0[:, 64:192]], axis=1)
    new_total_c = np.concatenate([cy, o0[:, 192:320]], axis=1)
    return hy, new_total_h, new_total_c
